# revision 14
# baseline (speedup 1.0000x reference)
"""Trainium2 Bass kernel: inclusive cumsum along L for X (4, 8192, 32, 32) f32.

Strategy (8 NeuronCores, SPMD), v5 — bf16 traffic, PE does only the scan:
  - Shard: core i gets b = i//2, c-half = i%2 -> a (8192, 512) slab, host-cast
    to bf16 (HBM per core: 8 MiB in + 8 MiB out, ~47 us DMA roofline).
  - The PE clock-gate sits at 4/8 (1.2 GHz) for non-dense matmul streams on
    this part (measured 535 ns per 512-col matmul), so the design keeps PE to
    ONE pass: per 128-row block i, yp_i = UT.T @ X_i in PSUM (UT = inclusive
    upper-triangular ones; matmul computes lhsT.T @ rhs).
  - Block sums ride for free: row 127 of yp_i IS colsum(X_i). A tiny
    PSUM->DRAM DMA extracts it (DMA has no partition-start restriction);
    a per-group DRAM->SBUF gather lands S[16, 512] in partition-major form.
  - Carries: per group of 16 blocks, 4 small matmuls compute
    T[16, C] = carry + exclusive-prefix(S) and the next carry [1, C]
    (the running carry lives at partition 0; engine APs must start at
    partition 0/32/64/96).
  - Carry injection: T rows are partition-broadcast by DMA (stride-0
    partition reads, SBUF->SBUF, no HBM traffic) into [128, 8*C] tiles; one
    DVE tensor_add per 1 MiB superblock applies them to the bf16 output
    tile after ScalarE/DVE copy the PSUM blocks out. No PE broadcast
    matmuls, no second PE pass.
  - Error budget (tolerance 2e-2 * max|out| ~ 9.1): bf16 input quantization
    random-walks to ~0.3; bf16 carry chain across 3 group boundaries ~2.7
    worst-case; T/output/add roundings ~0.9 each. Total ~5 worst-case.
"""

import numpy as np
import ml_dtypes
from contextlib import ExitStack

import concourse.bass as bass
import concourse.tile as tile
from concourse import bacc, masks, mybir
from concourse.bass_utils import run_bass_kernel_spmd

N_CORES = 8
B, L, D, N = 4, 8192, 32, 32
C_FULL = D * N          # 1024 columns per batch entry
C = C_FULL // 2         # 512 columns per core
P = 128                 # partitions / rows per scan block
NBLK = L // P           # 64 blocks per core
GBLK = 16               # blocks per carry group
NGRP = NBLK // GBLK     # 4 groups
SBB = 8                 # blocks per DMA superblock tile (1 MiB bf16)
NSB = NBLK // SBB       # 8 superblock tiles

_CACHE = {}


def _build_program():
    f32 = mybir.dt.float32
    bf16 = mybir.dt.bfloat16
    nc = bacc.Bacc(
        trn_type="TRN2", debug=False, num_devices=N_CORES, num_swdge_queues=2
    )
    x = nc.dram_tensor("x", [L, C], bf16, kind="ExternalInput").ap()
    y = nc.dram_tensor("y", [L, C], bf16, kind="ExternalOutput").ap()
    sdram = nc.dram_tensor("sdram", [NBLK, C], bf16, kind="Internal").ap()

    with tile.TileContext(nc) as tc, ExitStack() as ctx:
        const_pool = ctx.enter_context(tc.tile_pool(name="const", bufs=1))
        xin_pool = ctx.enter_context(tc.tile_pool(name="xin", bufs=5))
        yout_pool = ctx.enter_context(tc.tile_pool(name="yout", bufs=4))
        tbc_pool = ctx.enter_context(tc.tile_pool(name="tbc", bufs=3))
        small_pool = ctx.enter_context(tc.tile_pool(name="small", bufs=2))
        yps_pool = ctx.enter_context(tc.tile_pool(name="yps", bufs=6, space="PSUM"))
        tps_pool = ctx.enter_context(tc.tile_pool(name="tps", bufs=1, space="PSUM"))

        # UT: inclusive upper-triangular ones -> lhsT of the prefix matmul.
        ut = const_pool.tile([P, P], bf16, name="ut")
        masks.make_upper_triangular(nc, ut[:], 1.0, diag=True)
        # tmS: strict upper triangle -> exclusive prefix of block sums.
        tms = const_pool.tile([GBLK, GBLK], bf16, name="tms")
        masks.make_upper_triangular(nc, tms[:], 1.0, diag=False)
        ones_1x16 = const_pool.tile([1, GBLK], bf16, name="ones_1x16")
        nc.gpsimd.memset(ones_1x16[:], 1.0)
        ones_16x1 = const_pool.tile([GBLK, 1], bf16, name="ones_16x1")
        nc.gpsimd.memset(ones_16x1[:], 1.0)
        one_1x1 = const_pool.tile([1, 1], bf16, name="one_1x1")
        nc.gpsimd.memset(one_1x1[:], 1.0)
        ca0 = const_pool.tile([1, C], bf16, name="ca0")
        nc.gpsimd.memset(ca0[:], 0.0)

        prev_ca = ca0
        yts = {}

        def emit_group_blocks(g):
            """DMA in + UT scan matmuls + PSUM->SBUF copies for group g."""
            for sl in range(2):
                s = 2 * g + sl
                xt = xin_pool.tile([P, SBB * C], bf16, name=f"xt{s}", tag="xt", bufs=5)
                src = x[s * SBB * P : (s + 1) * SBB * P, :].rearrange(
                    "(ks p) c -> p ks c", p=P
                )
                dst = xt[:].rearrange("p (ks c) -> p ks c", ks=SBB)
                (nc.sync if s % 2 == 0 else nc.scalar).dma_start(out=dst, in_=src)
                yts[s] = (
                    xt,
                    yout_pool.tile([P, SBB * C], bf16, name=f"yt{s}", tag="yt", bufs=4),
                )
            for i in range(GBLK):
                blk = GBLK * g + i
                s, k = blk // SBB, blk % SBB
                xt, yt = yts[s]
                yp = yps_pool.tile([P, C], f32, name="yp", tag="yp", bufs=6)
                nc.tensor.matmul(
                    yp[:], ut[:], xt[:, k * C : (k + 1) * C], start=True, stop=True
                )
                if blk % 4 == 3:
                    nc.vector.tensor_copy(yt[:, k * C : (k + 1) * C], yp[:])
                else:
                    nc.scalar.copy(yt[:, k * C : (k + 1) * C], yp[:])
                # row 127 of the copied block = colsum(X_blk): stash to DRAM
                (nc.sync if blk % 2 == 0 else nc.gpsimd).dma_start(
                    out=sdram[blk : blk + 1, :],
                    in_=yt[127:128, k * C : (k + 1) * C],
                )

        def emit_group_carry(g):
            """Carry math + broadcast + apply + out-DMA for group g."""
            nonlocal prev_ca
            # gather this group's block sums into partition-major form
            sa = small_pool.tile([GBLK, C], bf16, name="sa", tag="sa", bufs=2)
            nc.sync.dma_start(out=sa[:], in_=sdram[GBLK * g : GBLK * (g + 1), :])
            ca = prev_ca
            # T[m] = carry + sum_{k<m} S[k]
            tp = tps_pool.tile([GBLK, C], f32, name="tp", tag="tp", bufs=1)
            nc.tensor.matmul(tp[:], ones_1x16[:], ca[:], start=True, stop=False)
            nc.tensor.matmul(tp[:], tms[:], sa[:], start=False, stop=True)
            tb = small_pool.tile([GBLK, C], bf16, name="tb", tag="tb", bufs=2)
            nc.vector.tensor_copy(tb[:], tp[:])
            if g < NGRP - 1:
                cp = tps_pool.tile([1, C], f32, name="cp", tag="cp", bufs=1)
                nc.tensor.matmul(cp[:], ones_16x1[:], sa[:], start=True, stop=False)
                nc.tensor.matmul(cp[:], one_1x1[:], ca[:], start=False, stop=True)
                nca = small_pool.tile([1, C], bf16, name="nca", tag="nca", bufs=2)
                nc.vector.tensor_copy(nca[:], cp[:])
                prev_ca = nca
            # broadcast T rows to all partitions (SBUF->SBUF stride-0 DMA),
            # then one add per superblock applies the carries.
            for sl in range(2):
                s = 2 * g + sl
                _, yt = yts[s]
                tbc = tbc_pool.tile([P, SBB * C], bf16, name=f"tbc{s}", tag="tbc", bufs=3)
                for k in range(SBB):
                    i = sl * SBB + k
                    nc.gpsimd.dma_start(
                        out=tbc[:, k * C : (k + 1) * C],
                        in_=tb[i : i + 1, :][:, None, :].broadcast_to([1, P, C]),
                    )
                nc.vector.tensor_add(yt[:], yt[:], tbc[:])
                ydst = y[s * SBB * P : (s + 1) * SBB * P, :].rearrange(
                    "(ks p) c -> p ks c", p=P
                )
                ysrc = yt[:].rearrange("p (ks c) -> p ks c", ks=SBB)
                (nc.scalar if s % 2 == 0 else nc.sync).dma_start(out=ydst, in_=ysrc)

        emit_group_blocks(0)
        for g in range(NGRP):
            if g + 1 < NGRP:
                emit_group_blocks(g + 1)
            emit_group_carry(g)

    nc.compile()
    return nc


def _get_program():
    if "nc" not in _CACHE:
        _CACHE["nc"] = _build_program()
    return _CACHE["nc"]


def _shard(X):
    """(4, 8192, 32, 32) f32 -> 8 contiguous (8192, 512) bf16 slabs."""
    Xv = X.reshape(B, L, C_FULL)
    shards = []
    for i in range(N_CORES):
        b, h = i // 2, i % 2
        shards.append(
            np.ascontiguousarray(Xv[b, :, h * C : (h + 1) * C]).astype(
                ml_dtypes.bfloat16
            )
        )
    return shards


def _unshard(parts):
    out = np.empty((B, L, C_FULL), dtype=np.float32)
    for i in range(N_CORES):
        b, h = i // 2, i % 2
        out[b, :, h * C : (h + 1) * C] = np.asarray(parts[i]).astype(np.float32)
    return out.reshape(B, L, D, N)


def kernel(X_in, _trace=False, _tmpdir=None, _trace_cores=None):
    X = np.asarray(X_in, dtype=np.float32)
    assert X.shape == (B, L, D, N), X.shape
    nc = _get_program()
    in_maps = [{"x": s} for s in _shard(X)]
    kwargs = {}
    if _trace:
        kwargs = dict(
            trace=True,
            tmpdir=_tmpdir,
            trace_cores=_trace_cores or list(range(N_CORES)),
        )
    res = run_bass_kernel_spmd(nc, in_maps, core_ids=list(range(N_CORES)), **kwargs)
    out = _unshard([res.results[i]["y"] for i in range(N_CORES)])
    kernel.last_results = res
    return out


# revision 15
# speedup vs baseline: 1.7516x; 1.7516x over previous
"""Trainium2 Bass kernel: inclusive cumsum along L for X (4, 8192, 32, 32) f32.

Strategy (8 NeuronCores, SPMD), v5 — bf16 traffic, PE does only the scan:
  - Shard: core i gets b = i//2, c-half = i%2 -> a (8192, 512) slab, host-cast
    to bf16 (HBM per core: 8 MiB in + 8 MiB out, ~47 us DMA roofline).
  - The PE clock-gate sits at 4/8 (1.2 GHz) for non-dense matmul streams on
    this part (measured 535 ns per 512-col matmul), so the design keeps PE to
    ONE pass: per 128-row block i, yp_i = UT.T @ X_i in PSUM (UT = inclusive
    upper-triangular ones; matmul computes lhsT.T @ rhs).
  - Block sums ride for free: row 127 of yp_i IS colsum(X_i). A tiny
    PSUM->DRAM DMA extracts it (DMA has no partition-start restriction);
    a per-group DRAM->SBUF gather lands S[16, 512] in partition-major form.
  - Carries: per group of 16 blocks, 4 small matmuls compute
    T[16, C] = carry + exclusive-prefix(S) and the next carry [1, C]
    (the running carry lives at partition 0; engine APs must start at
    partition 0/32/64/96).
  - Carry injection: T rows are partition-broadcast by DMA (stride-0
    partition reads, SBUF->SBUF, no HBM traffic) into [128, 8*C] tiles; one
    DVE tensor_add per 1 MiB superblock applies them to the bf16 output
    tile after ScalarE/DVE copy the PSUM blocks out. No PE broadcast
    matmuls, no second PE pass.
  - Error budget (tolerance 2e-2 * max|out| ~ 9.1): bf16 input quantization
    random-walks to ~0.3; bf16 carry chain across 3 group boundaries ~2.7
    worst-case; T/output/add roundings ~0.9 each. Total ~5 worst-case.
"""

import numpy as np
import ml_dtypes
from contextlib import ExitStack

import concourse.bass as bass
import concourse.tile as tile
from concourse import bacc, masks, mybir
from concourse.bass_utils import run_bass_kernel_spmd

N_CORES = 8
B, L, D, N = 4, 8192, 32, 32
C_FULL = D * N          # 1024 columns per batch entry
C = C_FULL // 2         # 512 columns per core
P = 128                 # partitions / rows per scan block
NBLK = L // P           # 64 blocks per core
GBLK = 16               # blocks per carry group
NGRP = NBLK // GBLK     # 4 groups
SBB = 8                 # blocks per DMA superblock tile (1 MiB bf16)
NSB = NBLK // SBB       # 8 superblock tiles

_CACHE = {}


def _build_program():
    f32 = mybir.dt.float32
    bf16 = mybir.dt.bfloat16
    nc = bacc.Bacc(
        trn_type="TRN2", debug=False, num_devices=N_CORES, num_swdge_queues=2
    )
    x = nc.dram_tensor("x", [L, C], bf16, kind="ExternalInput").ap()
    y = nc.dram_tensor("y", [L, C], bf16, kind="ExternalOutput").ap()
    sdram = nc.dram_tensor("sdram", [NBLK, C], bf16, kind="Internal").ap()

    with tile.TileContext(nc) as tc, ExitStack() as ctx:
        const_pool = ctx.enter_context(tc.tile_pool(name="const", bufs=1))
        xin_pool = ctx.enter_context(tc.tile_pool(name="xin", bufs=5))
        yout_pool = ctx.enter_context(tc.tile_pool(name="yout", bufs=4))
        tbc_pool = ctx.enter_context(tc.tile_pool(name="tbc", bufs=3))
        small_pool = ctx.enter_context(tc.tile_pool(name="small", bufs=2))
        yps_pool = ctx.enter_context(tc.tile_pool(name="yps", bufs=6, space="PSUM"))
        tps_pool = ctx.enter_context(tc.tile_pool(name="tps", bufs=1, space="PSUM"))

        # UT: inclusive upper-triangular ones -> lhsT of the prefix matmul.
        ut = const_pool.tile([P, P], bf16, name="ut")
        masks.make_upper_triangular(nc, ut[:], 1.0, diag=True)
        # tmS: strict upper triangle -> exclusive prefix of block sums.
        tms = const_pool.tile([GBLK, GBLK], bf16, name="tms")
        masks.make_upper_triangular(nc, tms[:], 1.0, diag=False)
        ones_1x16 = const_pool.tile([1, GBLK], bf16, name="ones_1x16")
        nc.gpsimd.memset(ones_1x16[:], 1.0)
        ones_16x1 = const_pool.tile([GBLK, 1], bf16, name="ones_16x1")
        nc.gpsimd.memset(ones_16x1[:], 1.0)
        one_1x1 = const_pool.tile([1, 1], bf16, name="one_1x1")
        nc.gpsimd.memset(one_1x1[:], 1.0)
        ca0 = const_pool.tile([1, C], bf16, name="ca0")
        nc.gpsimd.memset(ca0[:], 0.0)

        prev_ca = ca0
        yts = {}

        def emit_group_blocks(g):
            """DMA in + UT scan matmuls + PSUM->SBUF copies for group g."""
            for sl in range(2):
                s = 2 * g + sl
                xt = xin_pool.tile([P, SBB * C], bf16, name=f"xt{s}", tag="xt", bufs=5)
                src = x[s * SBB * P : (s + 1) * SBB * P, :].rearrange(
                    "(ks p) c -> p ks c", p=P
                )
                dst = xt[:].rearrange("p (ks c) -> p ks c", ks=SBB)
                (nc.sync if s % 2 == 0 else nc.scalar).dma_start(out=dst, in_=src)
                yts[s] = (
                    xt,
                    yout_pool.tile([P, SBB * C], bf16, name=f"yt{s}", tag="yt", bufs=4),
                )
            for i in range(GBLK):
                blk = GBLK * g + i
                s, k = blk // SBB, blk % SBB
                xt, yt = yts[s]
                yp = yps_pool.tile([P, C], f32, name="yp", tag="yp", bufs=6)
                nc.tensor.matmul(
                    yp[:], ut[:], xt[:, k * C : (k + 1) * C], start=True, stop=True
                )
                if blk % 4 == 3:
                    nc.vector.tensor_copy(yt[:, k * C : (k + 1) * C], yp[:])
                else:
                    nc.scalar.copy(yt[:, k * C : (k + 1) * C], yp[:])
                # row 127 of the copied block = colsum(X_blk): stash to DRAM
                (nc.sync if blk % 2 == 0 else nc.gpsimd).dma_start(
                    out=sdram[blk : blk + 1, :],
                    in_=yt[127:128, k * C : (k + 1) * C],
                )

        def emit_group_carry(g):
            """Carry math + broadcast + apply + out-DMA for group g."""
            nonlocal prev_ca
            # gather this group's block sums into partition-major form
            sa = small_pool.tile([GBLK, C], bf16, name="sa", tag="sa", bufs=2)
            nc.sync.dma_start(out=sa[:], in_=sdram[GBLK * g : GBLK * (g + 1), :])
            ca = prev_ca
            # T[m] = carry + sum_{k<m} S[k]
            tp = tps_pool.tile([GBLK, C], f32, name="tp", tag="tp", bufs=1)
            nc.tensor.matmul(tp[:], ones_1x16[:], ca[:], start=True, stop=False)
            nc.tensor.matmul(tp[:], tms[:], sa[:], start=False, stop=True)
            tb = small_pool.tile([GBLK, C], bf16, name="tb", tag="tb", bufs=2)
            nc.vector.tensor_copy(tb[:], tp[:])
            if g < NGRP - 1:
                cp = tps_pool.tile([1, C], f32, name="cp", tag="cp", bufs=1)
                nc.tensor.matmul(cp[:], ones_16x1[:], sa[:], start=True, stop=False)
                nc.tensor.matmul(cp[:], one_1x1[:], ca[:], start=False, stop=True)
                nca = small_pool.tile([1, C], bf16, name="nca", tag="nca", bufs=2)
                nc.vector.tensor_copy(nca[:], cp[:])
                prev_ca = nca
            # broadcast T rows to all partitions by log-doubling SBUF->SBUF
            # DMAs (each hop reads an ever-wider partition range, so the
            # reads stay port-parallel), then one add per superblock.
            for sl in range(2):
                s = 2 * g + sl
                _, yt = yts[s]
                tbc = tbc_pool.tile([P, SBB * C], bf16, name=f"tbc{s}", tag="tbc", bufs=3)
                seed_eng = nc.scalar if s % 2 == 0 else nc.sync
                seed_eng.dma_start(
                    out=tbc[0:1, :].rearrange("p (k c) -> p k c", k=SBB),
                    in_=tb[sl * SBB : (sl + 1) * SBB, :],
                )
                rows = 1
                while rows < P:
                    seed_eng.dma_start(
                        out=tbc[rows : 2 * rows, :], in_=tbc[0:rows, :]
                    )
                    rows *= 2
                nc.vector.tensor_add(yt[:], yt[:], tbc[:])
                ydst = y[s * SBB * P : (s + 1) * SBB * P, :].rearrange(
                    "(ks p) c -> p ks c", p=P
                )
                ysrc = yt[:].rearrange("p (ks c) -> p ks c", ks=SBB)
                (nc.scalar if s % 2 == 0 else nc.sync).dma_start(out=ydst, in_=ysrc)

        emit_group_blocks(0)
        for g in range(NGRP):
            if g + 1 < NGRP:
                emit_group_blocks(g + 1)
            emit_group_carry(g)

    nc.compile()
    return nc


def _get_program():
    if "nc" not in _CACHE:
        _CACHE["nc"] = _build_program()
    return _CACHE["nc"]


def _shard(X):
    """(4, 8192, 32, 32) f32 -> 8 contiguous (8192, 512) bf16 slabs."""
    Xv = X.reshape(B, L, C_FULL)
    shards = []
    for i in range(N_CORES):
        b, h = i // 2, i % 2
        shards.append(
            np.ascontiguousarray(Xv[b, :, h * C : (h + 1) * C]).astype(
                ml_dtypes.bfloat16
            )
        )
    return shards


def _unshard(parts):
    out = np.empty((B, L, C_FULL), dtype=np.float32)
    for i in range(N_CORES):
        b, h = i // 2, i % 2
        out[b, :, h * C : (h + 1) * C] = np.asarray(parts[i]).astype(np.float32)
    return out.reshape(B, L, D, N)


def kernel(X_in, _trace=False, _tmpdir=None, _trace_cores=None):
    X = np.asarray(X_in, dtype=np.float32)
    assert X.shape == (B, L, D, N), X.shape
    nc = _get_program()
    in_maps = [{"x": s} for s in _shard(X)]
    kwargs = {}
    if _trace:
        kwargs = dict(
            trace=True,
            tmpdir=_tmpdir,
            trace_cores=_trace_cores or list(range(N_CORES)),
        )
    res = run_bass_kernel_spmd(nc, in_maps, core_ids=list(range(N_CORES)), **kwargs)
    out = _unshard([res.results[i]["y"] for i in range(N_CORES)])
    kernel.last_results = res
    return out


# revision 16
# speedup vs baseline: 1.7778x; 1.0150x over previous
"""Trainium2 Bass kernel: inclusive cumsum along L for X (4, 8192, 32, 32) f32.

Strategy (8 NeuronCores, SPMD), v6 — bf16 traffic, PE does only the scan:
  - Shard: core i gets b = i//2, c-half = i%2 -> a (8192, 512) slab, host-cast
    to bf16 (HBM per core: 8 MiB in + 8 MiB out, ~47 us DMA roofline).
  - The host pre-arranges each slab into superblock-major form
    [8 superblocks, 128 partitions, 8 blocks * 512 cols] so every 1 MiB DMA
    is fully contiguous with 8 KiB per-partition runs (descriptor-count
    bound at 1 KiB runs halves effective DMA bandwidth).
  - The PE clock-gate sits at 4/8 (1.2 GHz) for non-dense matmul streams on
    this part (~535 ns per 512-col matmul), so PE does ONE pass: per
    128-row block i, yp_i = UT.T @ X_i (UT = inclusive upper-triangular
    ones; matmul computes lhsT.T @ rhs). ScalarE/DVE copy PSUM -> bf16.
  - Block sums ride for free: row 127 of each copied block IS colsum(X_i);
    all 8 land on partition 127 of the output tile -> ONE [1, 4096] DMA per
    superblock stashes them to a DRAM scratch; a per-group gather lands
    S[16, 512] partition-major.
  - Carries: per group of 16 blocks, 4 small matmuls compute
    T[16, C] = carry + exclusive-prefix(S) and the next carry [1, C] (the
    running carry lives at partition 0; engine APs must start at partition
    0/32/64/96).
  - Carry injection: per group, T rows are broadcast to all 128 partitions
    with a seed DMA + 7 log-doubling SBUF->SBUF DMAs (each hop reads an
    ever-wider partition range, keeping SBUF port reads parallel), then one
    DVE tensor_add per superblock applies them to the output tile. No PE
    broadcast matmuls, no second PE pass.
  - Error budget (tolerance 2e-2 * max|out| ~ 9.1): bf16 input quantization
    random-walks to ~0.3; bf16 carry chain across 3 group boundaries ~2.7
    worst-case; S/T/output/add roundings ~1 each. Total ~5 worst-case
    (measured ~3.6).
"""

import numpy as np
import ml_dtypes
from contextlib import ExitStack

import concourse.bass as bass
import concourse.tile as tile
from concourse import bacc, masks, mybir
from concourse.bass_utils import run_bass_kernel_spmd

N_CORES = 8
B, L, D, N = 4, 8192, 32, 32
C_FULL = D * N          # 1024 columns per batch entry
C = C_FULL // 2         # 512 columns per core
P = 128                 # partitions / rows per scan block
NBLK = L // P           # 64 blocks per core
GBLK = 16               # blocks per carry group
NGRP = NBLK // GBLK     # 4 groups
SBB = 8                 # blocks per DMA superblock tile (1 MiB bf16)
NSB = NBLK // SBB       # 8 superblock tiles
SBW = SBB * C           # free width of one superblock tile (4096)

_CACHE = {}


def _build_program():
    f32 = mybir.dt.float32
    bf16 = mybir.dt.bfloat16
    nc = bacc.Bacc(
        trn_type="TRN2", debug=False, num_devices=N_CORES, num_swdge_queues=2
    )
    # superblock-major, fully contiguous per superblock (host pre-arranged)
    x = nc.dram_tensor("x", [NSB, P, SBW], bf16, kind="ExternalInput").ap()
    y = nc.dram_tensor("y", [NSB, P, SBW], bf16, kind="ExternalOutput").ap()
    sdram = nc.dram_tensor("sdram", [NBLK, C], bf16, kind="Internal").ap()

    with tile.TileContext(nc) as tc, ExitStack() as ctx:
        const_pool = ctx.enter_context(tc.tile_pool(name="const", bufs=1))
        xin_pool = ctx.enter_context(tc.tile_pool(name="xin", bufs=5))
        yout_pool = ctx.enter_context(tc.tile_pool(name="yout", bufs=4))
        tbc_pool = ctx.enter_context(tc.tile_pool(name="tbc", bufs=2))
        small_pool = ctx.enter_context(tc.tile_pool(name="small", bufs=2))
        yps_pool = ctx.enter_context(tc.tile_pool(name="yps", bufs=6, space="PSUM"))
        tps_pool = ctx.enter_context(tc.tile_pool(name="tps", bufs=1, space="PSUM"))

        # UT: inclusive upper-triangular ones -> lhsT of the prefix matmul.
        ut = const_pool.tile([P, P], bf16, name="ut")
        masks.make_upper_triangular(nc, ut[:], 1.0, diag=True)
        # tmS: strict upper triangle -> exclusive prefix of block sums.
        tms = const_pool.tile([GBLK, GBLK], bf16, name="tms")
        masks.make_upper_triangular(nc, tms[:], 1.0, diag=False)
        ones_1x16 = const_pool.tile([1, GBLK], bf16, name="ones_1x16")
        nc.gpsimd.memset(ones_1x16[:], 1.0)
        ones_16x1 = const_pool.tile([GBLK, 1], bf16, name="ones_16x1")
        nc.gpsimd.memset(ones_16x1[:], 1.0)
        one_1x1 = const_pool.tile([1, 1], bf16, name="one_1x1")
        nc.gpsimd.memset(one_1x1[:], 1.0)
        ca0 = const_pool.tile([1, C], bf16, name="ca0")
        nc.gpsimd.memset(ca0[:], 0.0)

        prev_ca = ca0
        yts = {}

        def emit_group_blocks(g):
            """DMA in + UT scan matmuls + PSUM->SBUF copies for group g."""
            for sl in range(2):
                s = 2 * g + sl
                xt = xin_pool.tile([P, SBW], bf16, name=f"xt{s}", tag="xt", bufs=5)
                (nc.sync if s % 2 == 0 else nc.scalar).dma_start(
                    out=xt[:], in_=x[s]
                )
                yts[s] = (
                    xt,
                    yout_pool.tile([P, SBW], bf16, name=f"yt{s}", tag="yt", bufs=4),
                )
            for i in range(GBLK):
                blk = GBLK * g + i
                s, k = blk // SBB, blk % SBB
                xt, yt = yts[s]
                yp = yps_pool.tile([P, C], f32, name="yp", tag="yp", bufs=6)
                nc.tensor.matmul(
                    yp[:], ut[:], xt[:, k * C : (k + 1) * C], start=True, stop=True
                )
                if blk % 4 == 3:
                    nc.vector.tensor_copy(yt[:, k * C : (k + 1) * C], yp[:])
                else:
                    nc.scalar.copy(yt[:, k * C : (k + 1) * C], yp[:])
                if k == SBB - 1:
                    # rows 127 of the 8 copied blocks = their colsums; one
                    # [1, 4096] DMA stashes all of them to DRAM scratch.
                    nc.gpsimd.dma_start(
                        out=sdram[s * SBB : (s + 1) * SBB, :],
                        in_=yt[127:128, :],
                    )

        def emit_group_carry(g):
            """Carry math + broadcast + apply + out-DMA for group g."""
            nonlocal prev_ca
            # gather this group's block sums into partition-major form
            sa = small_pool.tile([GBLK, C], bf16, name="sa", tag="sa", bufs=2)
            nc.gpsimd.dma_start(out=sa[:], in_=sdram[GBLK * g : GBLK * (g + 1), :])
            ca = prev_ca
            # T[m] = carry + sum_{k<m} S[k]
            tp = tps_pool.tile([GBLK, C], f32, name="tp", tag="tp", bufs=1)
            nc.tensor.matmul(tp[:], ones_1x16[:], ca[:], start=True, stop=False)
            nc.tensor.matmul(tp[:], tms[:], sa[:], start=False, stop=True)
            tb = small_pool.tile([GBLK, C], bf16, name="tb", tag="tb", bufs=2)
            nc.vector.tensor_copy(tb[:], tp[:])
            if g < NGRP - 1:
                cp = tps_pool.tile([1, C], f32, name="cp", tag="cp", bufs=1)
                nc.tensor.matmul(cp[:], ones_16x1[:], sa[:], start=True, stop=False)
                nc.tensor.matmul(cp[:], one_1x1[:], ca[:], start=False, stop=True)
                nca = small_pool.tile([1, C], bf16, name="nca", tag="nca", bufs=2)
                nc.vector.tensor_copy(nca[:], cp[:])
                prev_ca = nca
            # broadcast all 16 T rows to all partitions: seed partition 0,
            # then log-double (each hop reads a widening partition range).
            tbc = tbc_pool.tile([P, 2 * SBW], bf16, name=f"tbc{g}", tag="tbc", bufs=2)
            eng = nc.scalar if g % 2 == 0 else nc.sync
            eng.dma_start(
                out=tbc[0:1, :].rearrange("p (k c) -> p k c", k=GBLK), in_=tb[:]
            )
            rows = 1
            while rows < P:
                eng.dma_start(out=tbc[rows : 2 * rows, :], in_=tbc[0:rows, :])
                rows *= 2
            for sl in range(2):
                s = 2 * g + sl
                _, yt = yts[s]
                nc.vector.tensor_add(
                    yt[:], yt[:], tbc[:, sl * SBW : (sl + 1) * SBW]
                )
                (nc.scalar if s % 2 == 0 else nc.sync).dma_start(
                    out=y[s], in_=yt[:]
                )

        emit_group_blocks(0)
        for g in range(NGRP):
            if g + 1 < NGRP:
                emit_group_blocks(g + 1)
            emit_group_carry(g)

    nc.compile()
    return nc


def _get_program():
    if "nc" not in _CACHE:
        _CACHE["nc"] = _build_program()
    return _CACHE["nc"]


def _shard(X):
    """(4, 8192, 32, 32) f32 -> 8 superblock-major (8, 128, 4096) bf16 slabs."""
    Xv = X.reshape(B, L, C_FULL)
    shards = []
    for i in range(N_CORES):
        b, h = i // 2, i % 2
        slab = Xv[b, :, h * C : (h + 1) * C]          # (8192, 512)
        arr = (
            slab.reshape(NSB, SBB, P, C)
            .transpose(0, 2, 1, 3)                     # (8, 128, 8, 512)
            .reshape(NSB, P, SBW)
        )
        shards.append(np.ascontiguousarray(arr).astype(ml_dtypes.bfloat16))
    return shards


def _unshard(parts):
    out = np.empty((B, L, C_FULL), dtype=np.float32)
    for i in range(N_CORES):
        b, h = i // 2, i % 2
        arr = np.asarray(parts[i]).astype(np.float32)  # (8, 128, 4096)
        slab = (
            arr.reshape(NSB, P, SBB, C)
            .transpose(0, 2, 1, 3)                     # (8, 8, 128, 512)
            .reshape(L, C)
        )
        out[b, :, h * C : (h + 1) * C] = slab
    return out.reshape(B, L, D, N)


def kernel(X_in, _trace=False, _tmpdir=None, _trace_cores=None):
    X = np.asarray(X_in, dtype=np.float32)
    assert X.shape == (B, L, D, N), X.shape
    nc = _get_program()
    in_maps = [{"x": s} for s in _shard(X)]
    kwargs = {}
    if _trace:
        kwargs = dict(
            trace=True,
            tmpdir=_tmpdir,
            trace_cores=_trace_cores or list(range(N_CORES)),
        )
    res = run_bass_kernel_spmd(nc, in_maps, core_ids=list(range(N_CORES)), **kwargs)
    out = _unshard([res.results[i]["y"] for i in range(N_CORES)])
    kernel.last_results = res
    return out


# revision 17
# speedup vs baseline: 1.8198x; 1.0236x over previous
"""Trainium2 Bass kernel: inclusive cumsum along L for X (4, 8192, 32, 32) f32.

Strategy (8 NeuronCores, SPMD), v6 — bf16 traffic, PE does only the scan:
  - Shard: core i gets b = i//2, c-half = i%2 -> a (8192, 512) slab, host-cast
    to bf16 (HBM per core: 8 MiB in + 8 MiB out, ~47 us DMA roofline).
  - The host pre-arranges each slab into superblock-major form
    [8 superblocks, 128 partitions, 8 blocks * 512 cols] so every 1 MiB DMA
    is fully contiguous with 8 KiB per-partition runs (descriptor-count
    bound at 1 KiB runs halves effective DMA bandwidth).
  - The PE clock-gate sits at 4/8 (1.2 GHz) for non-dense matmul streams on
    this part (~535 ns per 512-col matmul), so PE does ONE pass: per
    128-row block i, yp_i = UT.T @ X_i (UT = inclusive upper-triangular
    ones; matmul computes lhsT.T @ rhs). ScalarE/DVE copy PSUM -> bf16.
  - Block sums ride for free: row 127 of each copied block IS colsum(X_i);
    all 8 land on partition 127 of the output tile -> ONE [1, 4096] DMA per
    superblock stashes them to a DRAM scratch; a per-group gather lands
    S[16, 512] partition-major.
  - Carries: per group of 16 blocks, 4 small matmuls compute
    T[16, C] = carry + exclusive-prefix(S) and the next carry [1, C] (the
    running carry lives at partition 0; engine APs must start at partition
    0/32/64/96).
  - Carry injection: per group, T rows are broadcast to all 128 partitions
    with a seed DMA + 7 log-doubling SBUF->SBUF DMAs (each hop reads an
    ever-wider partition range, keeping SBUF port reads parallel), then one
    DVE tensor_add per superblock applies them to the output tile. No PE
    broadcast matmuls, no second PE pass.
  - Error budget (tolerance 2e-2 * max|out| ~ 9.1): bf16 input quantization
    random-walks to ~0.3; bf16 carry chain across 3 group boundaries ~2.7
    worst-case; S/T/output/add roundings ~1 each. Total ~5 worst-case
    (measured ~3.6).
"""

import numpy as np
import ml_dtypes
from contextlib import ExitStack

import concourse.bass as bass
import concourse.tile as tile
from concourse import bacc, masks, mybir
from concourse.bass_utils import run_bass_kernel_spmd

N_CORES = 8
B, L, D, N = 4, 8192, 32, 32
C_FULL = D * N          # 1024 columns per batch entry
C = C_FULL // 2         # 512 columns per core
P = 128                 # partitions / rows per scan block
NBLK = L // P           # 64 blocks per core
GBLK = 16               # blocks per carry group
NGRP = NBLK // GBLK     # 4 groups
SBB = 8                 # blocks per DMA superblock tile (1 MiB bf16)
NSB = NBLK // SBB       # 8 superblock tiles
SBW = SBB * C           # free width of one superblock tile (4096)

_CACHE = {}


def _build_program():
    f32 = mybir.dt.float32
    bf16 = mybir.dt.bfloat16
    nc = bacc.Bacc(
        trn_type="TRN2", debug=False, num_devices=N_CORES, num_swdge_queues=2
    )
    # superblock-major, fully contiguous per superblock (host pre-arranged)
    x = nc.dram_tensor("x", [NSB, P, SBW], bf16, kind="ExternalInput").ap()
    y = nc.dram_tensor("y", [NSB, P, SBW], bf16, kind="ExternalOutput").ap()
    sdram = nc.dram_tensor("sdram", [NBLK, C], bf16, kind="Internal").ap()

    with tile.TileContext(nc) as tc, ExitStack() as ctx:
        const_pool = ctx.enter_context(tc.tile_pool(name="const", bufs=1))
        xin_pool = ctx.enter_context(tc.tile_pool(name="xin", bufs=5))
        yout_pool = ctx.enter_context(tc.tile_pool(name="yout", bufs=4))
        tbc_pool = ctx.enter_context(tc.tile_pool(name="tbc", bufs=2))
        small_pool = ctx.enter_context(tc.tile_pool(name="small", bufs=2))
        yps_pool = ctx.enter_context(tc.tile_pool(name="yps", bufs=6, space="PSUM"))
        tps_pool = ctx.enter_context(tc.tile_pool(name="tps", bufs=1, space="PSUM"))

        # UT: inclusive upper-triangular ones -> lhsT of the prefix matmul.
        ut = const_pool.tile([P, P], bf16, name="ut")
        masks.make_upper_triangular(nc, ut[:], 1.0, diag=True)
        # tmS: strict upper triangle -> exclusive prefix of block sums.
        tms = const_pool.tile([GBLK, GBLK], bf16, name="tms")
        masks.make_upper_triangular(nc, tms[:], 1.0, diag=False)
        ones_1x16 = const_pool.tile([1, GBLK], bf16, name="ones_1x16")
        nc.gpsimd.memset(ones_1x16[:], 1.0)
        ones_16x1 = const_pool.tile([GBLK, 1], bf16, name="ones_16x1")
        nc.gpsimd.memset(ones_16x1[:], 1.0)
        one_1x1 = const_pool.tile([1, 1], bf16, name="one_1x1")
        nc.gpsimd.memset(one_1x1[:], 1.0)
        ca0 = const_pool.tile([1, C], bf16, name="ca0")
        nc.gpsimd.memset(ca0[:], 0.0)

        prev_ca = ca0
        yts = {}

        def emit_group_blocks(g):
            """DMA in + UT scan matmuls + PSUM->SBUF copies for group g."""
            for sl in range(2):
                s = 2 * g + sl
                xt = xin_pool.tile([P, SBW], bf16, name=f"xt{s}", tag="xt", bufs=5)
                (nc.sync if s % 2 == 0 else nc.scalar).dma_start(
                    out=xt[:], in_=x[s]
                )
                yts[s] = (
                    xt,
                    yout_pool.tile([P, SBW], bf16, name=f"yt{s}", tag="yt", bufs=4),
                )
            for i in range(GBLK):
                blk = GBLK * g + i
                s, k = blk // SBB, blk % SBB
                xt, yt = yts[s]
                yp = yps_pool.tile([P, C], f32, name="yp", tag="yp", bufs=6)
                nc.tensor.matmul(
                    yp[:], ut[:], xt[:, k * C : (k + 1) * C], start=True, stop=True
                )
                if blk % 4 == 3:
                    nc.vector.tensor_copy(yt[:, k * C : (k + 1) * C], yp[:])
                else:
                    nc.scalar.copy(yt[:, k * C : (k + 1) * C], yp[:])
                if k == SBB - 1:
                    # rows 127 of the 8 copied blocks = their colsums; one
                    # [1, 4096] DMA stashes all of them to DRAM scratch.
                    nc.gpsimd.dma_start(
                        out=sdram[s * SBB : (s + 1) * SBB, :],
                        in_=yt[127:128, :],
                    )

        def emit_group_carry(g):
            """Carry math + broadcast + apply + out-DMA for group g."""
            nonlocal prev_ca
            # gather this group's block sums into partition-major form
            sa = small_pool.tile([GBLK, C], bf16, name="sa", tag="sa", bufs=2)
            nc.gpsimd.dma_start(out=sa[:], in_=sdram[GBLK * g : GBLK * (g + 1), :])
            ca = prev_ca
            # T[m] = carry + sum_{k<m} S[k]
            tp = tps_pool.tile([GBLK, C], f32, name="tp", tag="tp", bufs=1)
            nc.tensor.matmul(tp[:], ones_1x16[:], ca[:], start=True, stop=False)
            nc.tensor.matmul(tp[:], tms[:], sa[:], start=False, stop=True)
            tb = small_pool.tile([GBLK, C], bf16, name="tb", tag="tb", bufs=2)
            nc.vector.tensor_copy(tb[:], tp[:])
            if g < NGRP - 1:
                cp = tps_pool.tile([1, C], f32, name="cp", tag="cp", bufs=1)
                nc.tensor.matmul(cp[:], ones_16x1[:], sa[:], start=True, stop=False)
                nc.tensor.matmul(cp[:], one_1x1[:], ca[:], start=False, stop=True)
                nca = small_pool.tile([1, C], bf16, name="nca", tag="nca", bufs=2)
                nc.vector.tensor_copy(nca[:], cp[:])
                prev_ca = nca
            # broadcast all 16 T rows to all partitions: 4 parallel seeds on
            # partitions 0-3, then log-double (each hop reads a widening
            # partition range, keeping SBUF port reads parallel). Each
            # group's chain lives on its own DMA queue to avoid
            # head-of-line blocking between chains and the big transfers.
            tbc = tbc_pool.tile([P, 2 * SBW], bf16, name=f"tbc{g}", tag="tbc", bufs=2)
            eng = (nc.gpsimd, nc.sync, nc.scalar, nc.gpsimd)[g]
            for r in range(4):
                eng.dma_start(
                    out=tbc[r : r + 1, :].rearrange("p (k c) -> p k c", k=GBLK),
                    in_=tb[:],
                )
            rows = 4
            while rows < P:
                eng.dma_start(out=tbc[rows : 2 * rows, :], in_=tbc[0:rows, :])
                rows *= 2
            for sl in range(2):
                s = 2 * g + sl
                _, yt = yts[s]
                nc.vector.tensor_add(
                    yt[:], yt[:], tbc[:, sl * SBW : (sl + 1) * SBW]
                )
                (nc.scalar if s % 2 == 0 else nc.sync).dma_start(
                    out=y[s], in_=yt[:]
                )

        emit_group_blocks(0)
        for g in range(NGRP):
            if g + 1 < NGRP:
                emit_group_blocks(g + 1)
            emit_group_carry(g)

    nc.compile()
    return nc


def _get_program():
    if "nc" not in _CACHE:
        _CACHE["nc"] = _build_program()
    return _CACHE["nc"]


def _shard(X):
    """(4, 8192, 32, 32) f32 -> 8 superblock-major (8, 128, 4096) bf16 slabs."""
    Xv = X.reshape(B, L, C_FULL)
    shards = []
    for i in range(N_CORES):
        b, h = i // 2, i % 2
        slab = Xv[b, :, h * C : (h + 1) * C]          # (8192, 512)
        arr = (
            slab.reshape(NSB, SBB, P, C)
            .transpose(0, 2, 1, 3)                     # (8, 128, 8, 512)
            .reshape(NSB, P, SBW)
        )
        shards.append(np.ascontiguousarray(arr).astype(ml_dtypes.bfloat16))
    return shards


def _unshard(parts):
    out = np.empty((B, L, C_FULL), dtype=np.float32)
    for i in range(N_CORES):
        b, h = i // 2, i % 2
        arr = np.asarray(parts[i]).astype(np.float32)  # (8, 128, 4096)
        slab = (
            arr.reshape(NSB, P, SBB, C)
            .transpose(0, 2, 1, 3)                     # (8, 8, 128, 512)
            .reshape(L, C)
        )
        out[b, :, h * C : (h + 1) * C] = slab
    return out.reshape(B, L, D, N)


def kernel(X_in, _trace=False, _tmpdir=None, _trace_cores=None):
    X = np.asarray(X_in, dtype=np.float32)
    assert X.shape == (B, L, D, N), X.shape
    nc = _get_program()
    in_maps = [{"x": s} for s in _shard(X)]
    kwargs = {}
    if _trace:
        kwargs = dict(
            trace=True,
            tmpdir=_tmpdir,
            trace_cores=_trace_cores or list(range(N_CORES)),
        )
    res = run_bass_kernel_spmd(nc, in_maps, core_ids=list(range(N_CORES)), **kwargs)
    out = _unshard([res.results[i]["y"] for i in range(N_CORES)])
    kernel.last_results = res
    return out


# revision 18
# speedup vs baseline: 2.9604x; 1.6268x over previous
"""Variant A: 3-pass PE (colsum + carry-broadcast + scan matmuls), minimal DMA.

Same sharding/host-arrangement as v6/v7 (superblock-major contiguous bf16).
Only 16 DMAs total (8 in + 8 out). All carry machinery stays on the PE:
  - phase 1: per group of 16 blocks, one-hot-column matmuls accumulate
    block column-sums S[16, C] into one PSUM bank;
  - phase 2: 4 small matmuls produce T[16, C] (carry + exclusive prefix)
    and the next carry at partition 0;
  - phase 3: per block, a row-selector matmul broadcasts T_i into PSUM
    (start=True), the UT matmul accumulates the in-block prefix, and
    ScalarE/DVE copy PSUM -> bf16 output tiles.
The PE stream is nearly gap-free; if the HAM clock-gate warms this runs at
2.4 GHz (~52 us PE), at 1.2 GHz it is ~103 us.
"""

import numpy as np
import ml_dtypes
from contextlib import ExitStack

import concourse.bass as bass
import concourse.tile as tile
from concourse import bacc, masks, mybir
from concourse.bass_utils import run_bass_kernel_spmd

N_CORES = 8
B, L, D, N = 4, 8192, 32, 32
C_FULL = D * N
C = C_FULL // 2
P = 128
NBLK = L // P
GBLK = 16
NGRP = NBLK // GBLK
SBB = 8
NSB = NBLK // SBB
SBW = SBB * C

_CACHE = {}


def _build_program():
    f32 = mybir.dt.float32
    bf16 = mybir.dt.bfloat16
    nc = bacc.Bacc(
        trn_type="TRN2", debug=False, num_devices=N_CORES, num_swdge_queues=2
    )
    x = nc.dram_tensor("x", [NSB, P, SBW], bf16, kind="ExternalInput").ap()
    y = nc.dram_tensor("y", [NSB, P, SBW], bf16, kind="ExternalOutput").ap()

    with tile.TileContext(nc) as tc, ExitStack() as ctx:
        const_pool = ctx.enter_context(tc.tile_pool(name="const", bufs=1))
        xin_pool = ctx.enter_context(tc.tile_pool(name="xin", bufs=5))
        yout_pool = ctx.enter_context(tc.tile_pool(name="yout", bufs=4))
        small_pool = ctx.enter_context(tc.tile_pool(name="small", bufs=2))
        yps_pool = ctx.enter_context(tc.tile_pool(name="yps", bufs=4, space="PSUM"))
        sps_pool = ctx.enter_context(tc.tile_pool(name="sps", bufs=2, space="PSUM"))
        tps_pool = ctx.enter_context(tc.tile_pool(name="tps", bufs=1, space="PSUM"))

        ut = const_pool.tile([P, P], bf16, name="ut")
        masks.make_upper_triangular(nc, ut[:], 1.0, diag=True)
        # Z1Z: ones in column GBLK-1; a 16-wide slice puts the ones-column
        # at any position 0..15 (phase-1 one-hot stationaries).
        z1z = const_pool.tile([P, 2 * GBLK - 1], bf16, name="z1z")
        nc.gpsimd.memset(z1z[:], 0.0)
        nc.gpsimd.memset(z1z[:, GBLK - 1 : GBLK], 1.0)
        # RZ row-selector bank: slice [:, i*128:(i+1)*128] is all-ones in
        # row i -> matmul replicates T row i onto all 128 output partitions.
        rz = const_pool.tile([GBLK, GBLK * P], bf16, name="rz")
        nc.gpsimd.memset(rz[:], 1.0)
        nc.gpsimd.affine_select(
            out=rz[:], in_=rz[:], compare_op=mybir.AluOpType.is_ge,
            fill=0.0, base=0, pattern=[[1, GBLK * P]], channel_multiplier=-P,
        )
        nc.gpsimd.affine_select(
            out=rz[:], in_=rz[:], compare_op=mybir.AluOpType.is_ge,
            fill=0.0, base=P - 1, pattern=[[-1, GBLK * P]], channel_multiplier=P,
        )
        tms = const_pool.tile([GBLK, GBLK], bf16, name="tms")
        masks.make_upper_triangular(nc, tms[:], 1.0, diag=False)
        ones_1x16 = const_pool.tile([1, GBLK], bf16, name="ones_1x16")
        nc.gpsimd.memset(ones_1x16[:], 1.0)
        ones_16x1 = const_pool.tile([GBLK, 1], bf16, name="ones_16x1")
        nc.gpsimd.memset(ones_16x1[:], 1.0)
        one_1x1 = const_pool.tile([1, 1], bf16, name="one_1x1")
        nc.gpsimd.memset(one_1x1[:], 1.0)
        ca0 = const_pool.tile([1, C], bf16, name="ca0")
        nc.gpsimd.memset(ca0[:], 0.0)

        prev_ca = ca0
        xts = {}

        def emit_phase1(g):
            for sl in range(2):
                s = 2 * g + sl
                xt = xin_pool.tile([P, SBW], bf16, name=f"xt{s}", tag="xt", bufs=5)
                (nc.sync if s % 2 == 0 else nc.scalar).dma_start(out=xt[:], in_=x[s])
                xts[s] = xt
            sp = sps_pool.tile([GBLK, C], f32, name="sp", tag="sp", bufs=2)
            for i in range(GBLK):
                blk = GBLK * g + i
                s, k = blk // SBB, blk % SBB
                nc.tensor.matmul(
                    sp[:],
                    z1z[:, GBLK - 1 - i : 2 * GBLK - 1 - i],
                    xts[s][:, k * C : (k + 1) * C],
                    start=(i == 0),
                    stop=(i == GBLK - 1),
                )
            return sp

        def emit_carry_math(g, sp):
            nonlocal prev_ca
            sa = small_pool.tile([GBLK, C], bf16, name="sa", tag="sa", bufs=2)
            nc.vector.tensor_copy(sa[:], sp[:])
            ca = prev_ca
            tp = tps_pool.tile([GBLK, C], f32, name="tp", tag="tp", bufs=1)
            nc.tensor.matmul(tp[:], ones_1x16[:], ca[:], start=True, stop=False)
            nc.tensor.matmul(tp[:], tms[:], sa[:], start=False, stop=True)
            tb = small_pool.tile([GBLK, C], bf16, name="tb", tag="tb", bufs=2)
            nc.vector.tensor_copy(tb[:], tp[:])
            if g < NGRP - 1:
                cp = tps_pool.tile([1, C], f32, name="cp", tag="cp", bufs=1)
                nc.tensor.matmul(cp[:], ones_16x1[:], sa[:], start=True, stop=False)
                nc.tensor.matmul(cp[:], one_1x1[:], ca[:], start=False, stop=True)
                nca = small_pool.tile([1, C], bf16, name="nca", tag="nca", bufs=2)
                nc.vector.tensor_copy(nca[:], cp[:])
                prev_ca = nca
            return tb

        def emit_phase3(g, tb):
            yt = None
            for i in range(GBLK):
                blk = GBLK * g + i
                s, k = blk // SBB, blk % SBB
                if k == 0:
                    yt = yout_pool.tile([P, SBW], bf16, name=f"yt{s}", tag="yt", bufs=4)
                yp = yps_pool.tile([P, C], f32, name="yp", tag="yp", bufs=4)
                nc.tensor.matmul(
                    yp[:], rz[:, i * P : (i + 1) * P], tb[:], start=True, stop=False
                )
                nc.tensor.matmul(
                    yp[:], ut[:], xts[s][:, k * C : (k + 1) * C],
                    start=False, stop=True,
                )
                if blk % 4 == 3:
                    nc.vector.tensor_copy(yt[:, k * C : (k + 1) * C], yp[:])
                else:
                    nc.scalar.copy(yt[:, k * C : (k + 1) * C], yp[:])
                if k == SBB - 1:
                    (nc.scalar if s % 2 == 0 else nc.sync).dma_start(
                        out=y[s], in_=yt[:]
                    )

        # schedule: ph_0, ph_1, T_0, p3_0, ph_2, T_1, p3_1, ph_3, T_2, p3_2, T_3, p3_3
        sps = {}
        tbs = {}
        sps[0] = emit_phase1(0)
        sps[1] = emit_phase1(1)
        tbs[0] = emit_carry_math(0, sps[0])
        emit_phase3(0, tbs[0])
        sps[2] = emit_phase1(2)
        tbs[1] = emit_carry_math(1, sps[1])
        emit_phase3(1, tbs[1])
        sps[3] = emit_phase1(3)
        tbs[2] = emit_carry_math(2, sps[2])
        emit_phase3(2, tbs[2])
        tbs[3] = emit_carry_math(3, sps[3])
        emit_phase3(3, tbs[3])

    nc.compile()
    return nc


def _get_program():
    if "nc" not in _CACHE:
        _CACHE["nc"] = _build_program()
    return _CACHE["nc"]


def _shard(X):
    Xv = X.reshape(B, L, C_FULL)
    shards = []
    for i in range(N_CORES):
        b, h = i // 2, i % 2
        slab = Xv[b, :, h * C : (h + 1) * C]
        arr = (
            slab.reshape(NSB, SBB, P, C).transpose(0, 2, 1, 3).reshape(NSB, P, SBW)
        )
        shards.append(np.ascontiguousarray(arr).astype(ml_dtypes.bfloat16))
    return shards


def _unshard(parts):
    out = np.empty((B, L, C_FULL), dtype=np.float32)
    for i in range(N_CORES):
        b, h = i // 2, i % 2
        arr = np.asarray(parts[i]).astype(np.float32)
        slab = arr.reshape(NSB, P, SBB, C).transpose(0, 2, 1, 3).reshape(L, C)
        out[b, :, h * C : (h + 1) * C] = slab
    return out.reshape(B, L, D, N)


def kernel(X_in, _trace=False, _tmpdir=None, _trace_cores=None):
    X = np.asarray(X_in, dtype=np.float32)
    assert X.shape == (B, L, D, N), X.shape
    nc = _get_program()
    in_maps = [{"x": s} for s in _shard(X)]
    kwargs = {}
    if _trace:
        kwargs = dict(
            trace=True,
            tmpdir=_tmpdir,
            trace_cores=_trace_cores or list(range(N_CORES)),
        )
    res = run_bass_kernel_spmd(nc, in_maps, core_ids=list(range(N_CORES)), **kwargs)
    out = _unshard([res.results[i]["y"] for i in range(N_CORES)])
    kernel.last_results = res
    return out
